# revision 25
# baseline (speedup 1.0000x reference)
"""Multi-head self-attention Trainium2 kernel (8 NeuronCores, SPMD).

Problem: x[2,2048,1024] f32, 16 heads x 64 dim, full QKV+attention+output
projection. Sharding: core = (batch n, head-group of 4 heads). Each core
computes partial^T = Wo_rows^T @ head_out^T for its 4 heads; host sums the
4 partials per batch and transposes back.

Device-side layout is fully "transposed" (feature dim on partitions):
  xT [1024, 2048]  ->  QT/KT [256, 2048] (d' on partitions)
                       V  [2048, 4*65]   (token on partitions, per-head
                                          [V_h | ones] for fused softmax sums)
  scoresT [keys, q] = KT_h^T-slices @ QT_h  (contraction over d=64)
  exp (no max subtraction: scores ~ N(0,1), |s| < ~12 is safe in f32)
  mask applied multiplicatively after exp (masked entries exp*0)
  PV: psum[65, Nq] = V'_h^T @ expT, row 64 = softmax denominators
  out^T/sums -> HO [256, 2048] -> partial^T [1024, 2048] = Wo^T @ HO

Heads are processed in interleaved pairs so the PE always has an
independent matmul chain while the other head's softmax runs (keeps the
HAM clock gate warm). Matmul dtype is float32r (TF32-like, full PE rate).
"""

import os
import sys
import numpy as np

if "/opt/trn_rl_repo" not in sys.path:
    sys.path.insert(0, "/opt/trn_rl_repo")

import ml_dtypes  # noqa: E402
import concourse.bass as bass  # noqa: E402
import concourse.mybir as mybir  # noqa: E402
from concourse import tile  # noqa: E402
from concourse import bacc  # noqa: E402
from concourse.bass_utils import run_bass_kernel_spmd  # noqa: E402
from contextlib import ExitStack  # noqa: E402

F32 = mybir.dt.float32
F32R = mybir.dt.float32r
BF16 = mybir.dt.bfloat16
AF = mybir.ActivationFunctionType

N, S, D = 2, 2048, 1024
H, HD = 16, 64
NCORES = 8
CORES_PER_BATCH = 4
HPC = H // CORES_PER_BATCH      # 4 heads per core
DPC = HPC * HD                  # 256 proj cols per core
NQ = 512                        # query block
NQB = S // NQ                   # 4 query blocks
KC = S // 128                   # 16 key chunks
DC = D // 128                   # 8 contraction chunks of embed dim

# modes: f32r (all f32r), mixed (f32r + bf16 attention weights), bf16, f32
MODE = os.environ.get("ATT_MODE", "mixed")
_d = {
    "f32r":  (F32R, F32R, F32R),
    "mixed": (F32R, BF16, F32R),
    "bf16":  (BF16, BF16, BF16),
    "f32":   (F32, F32, F32),
}
IO_DT, PV_DT, WO_DT = _d[MODE]
IO_NP = ml_dtypes.bfloat16 if IO_DT == BF16 else np.float32
WO_NP = ml_dtypes.bfloat16 if WO_DT == BF16 else np.float32
MASK_DT = BF16
MASK_NP = ml_dtypes.bfloat16
# recip/broadcast stays plain f32: the custom-DVE reciprocal can't tag
# its output as f32r for the BIR verifier, so the rank-1 broadcast matmul
# runs as a plain-f32 matmul (tiny; 4 cyc/row is irrelevant here)
RECIP_DT = F32


def _memset1(nc, ap):
    # DVE memset can't encode f32r; write 1.0 through an f32 view
    # (1.0 is exact in any truncated-mantissa f32 variant)
    if ap.dtype == F32R:
        ap = ap.bitcast(F32)
    nc.vector.memset(ap, 1.0)


def build_nc(with_bias: bool = True) -> bass.Bass:
    nc = bacc.Bacc()
    xT = nc.dram_tensor("xT", [D, S], IO_DT, kind="ExternalInput")
    maskT = nc.dram_tensor("maskT", [S, S], MASK_DT, kind="ExternalInput")
    wq = nc.dram_tensor("wq", [D, DPC], IO_DT, kind="ExternalInput")
    wk = nc.dram_tensor("wk", [D, DPC], IO_DT, kind="ExternalInput")
    wv = nc.dram_tensor("wv", [D, DPC], IO_DT, kind="ExternalInput")
    wo = nc.dram_tensor("wo", [DPC, D], WO_DT, kind="ExternalInput")
    if with_bias:
        bq = nc.dram_tensor("bq", [1, DPC], IO_DT, kind="ExternalInput")
        bk = nc.dram_tensor("bk", [1, DPC], IO_DT, kind="ExternalInput")
        bv = nc.dram_tensor("bv", [1, DPC], IO_DT, kind="ExternalInput")
    out = nc.dram_tensor("out", [D, S], F32, kind="ExternalOutput")

    with tile.TileContext(nc) as tc, ExitStack() as ctx:
        consts = ctx.enter_context(tc.tile_pool(name="consts", bufs=1))
        qkv_pool = ctx.enter_context(tc.tile_pool(name="qkv", bufs=1))

        if with_bias:
            ones_sb = consts.tile([1, S], IO_DT, tag="ones")
            _memset1(nc, ones_sb[:])


        # persistent activations. QT/KT are per-head [128, S] tiles: head
        # h's 64 d'-rows live at their natural partition offset, the other
        # 64 rows are zeroed so scores matmuls contract over K=128 (half-
        # array K=64 matmuls leave the HAM activity monitor cold -> 1.2GHz)
        QT = [qkv_pool.tile([128, S], IO_DT, tag=f"qt{h}", name=f"qt{h}")
              for h in range(HPC)]
        KT = [qkv_pool.tile([128, S], IO_DT, tag=f"kt{h}", name=f"kt{h}")
              for h in range(HPC)]
        for h in range(HPC):
            r0 = (HD * h) % 128
            rz = 64 - r0  # start of the unused half
            zq = QT[h][rz:rz + HD, :]
            zk = KT[h][rz:rz + HD, :]
            if IO_DT == F32R:
                zq, zk = zq.bitcast(F32), zk.bitcast(F32)
            nc.vector.memset(zq, 0.0)
            nc.vector.memset(zk, 0.0)
        V = [qkv_pool.tile([128, HPC * (HD + 1)], PV_DT, tag=f"v{t}",
                           name=f"v{t}") for t in range(KC)]
        HO = [qkv_pool.tile([128, S], WO_DT, tag=f"ho{m}", name=f"ho{m}")
              for m in range(2)]
        WO = [qkv_pool.tile([128, D], WO_DT, tag=f"wo{m}", name=f"wo{m}")
              for m in range(2)]
        for m in range(2):
            nc.sync.dma_start(WO[m][:], wo[128 * m:128 * (m + 1), :])

        # ---- phase 1: projections (scoped pools so SBUF frees after) ----
        with tc.tile_pool(name="ph1", bufs=1) as ph1, \
             tc.tile_pool(name="ph1ps", bufs=2, space="PSUM") as ph1ps:
            xt = [ph1.tile([128, S], IO_DT, tag=f"xt{i}", name=f"xt{i}")
                  for i in range(DC)]
            w_sb = {}
            for wname in ("wq", "wk", "wv"):
                w_sb[wname] = [
                    ph1.tile([128, DPC], IO_DT, tag=f"{wname}{i}",
                             name=f"{wname}{i}") for i in range(DC)]
            for i in range(DC):
                nc.sync.dma_start(xt[i][:], xT[128 * i:128 * (i + 1), :])
                for wname, wdram in (("wq", wq), ("wk", wk), ("wv", wv)):
                    nc.sync.dma_start(w_sb[wname][i][:],
                                      wdram[128 * i:128 * (i + 1), :])
            b_sb = {}
            if with_bias:
                for bname, bdram in (("bq", bq), ("bk", bk), ("bv", bv)):
                    b_sb[bname] = consts.tile([1, DPC], IO_DT, tag=bname,
                                              name=f"{bname}_sb")
                    nc.sync.dma_start(b_sb[bname][:], bdram[:])

            # QT / KT: out[d' tile, tok] = W-chunk^T @ xT-chunk.
            # t-blocks inner with fixed weights so LDWEIGHTS amortizes.
            for dst, wname, bname in ((QT, "wq", "bq"), (KT, "wk", "bk")):
                for m in range(2):
                    pss = [ph1ps.tile([128, NQ], F32, tag=f"projps{t}",
                                      bufs=1, name=f"projps{t}")
                           for t in range(NQB)]
                    for i in range(DC):
                        for t in range(NQB):
                            nc.tensor.matmul(
                                pss[t][:],
                                w_sb[wname][i][:, 128 * m:128 * (m + 1)],
                                xt[i][:, NQ * t:NQ * (t + 1)],
                                start=(i == 0),
                                stop=(not with_bias and i == DC - 1))
                    for t in range(NQB):
                        if with_bias:
                            nc.tensor.matmul(
                                pss[t][:],
                                b_sb[bname][:, 128 * m:128 * (m + 1)],
                                ones_sb[:, NQ * t:NQ * (t + 1)],
                                start=False, stop=True)
                        t_sl = slice(NQ * t, NQ * (t + 1))
                        nc.scalar.copy(dst[2 * m][0:HD, t_sl],
                                       pss[t][0:HD, :])
                        nc.scalar.copy(dst[2 * m + 1][HD:128, t_sl],
                                       pss[t][HD:128, :])

            # V natural: out[tok, d'] = xT-chunk^T(as lhsT) @ Wv-chunk
            for t in range(KC):
                ps = ph1ps.tile([128, DPC], F32, tag="vps", bufs=2,
                                name="vps")
                for i in range(DC):
                    nc.tensor.matmul(
                        ps[:],
                        xt[i][:, 128 * t:128 * (t + 1)],
                        w_sb["wv"][i][:],
                        start=(i == 0),
                        stop=(not with_bias and i == DC - 1))
                if with_bias:
                    nc.tensor.matmul(
                        ps[:], ones_sb[:, 128 * t:128 * (t + 1)],
                        b_sb["bv"][:], start=False, stop=True)
                v3 = V[t].rearrange("p (h d) -> p h d", d=HD + 1)
                nc.scalar.copy(v3[:, :, 0:HD],
                               ps.rearrange("p (h d) -> p h d", d=HD))
                _memset1(nc, v3[:, :, HD:HD + 1])

        # ---- phase 2+3: attention + output projection, per query block ----
        # (pools opened only after phase-1 pools release their SBUF/PSUM)
        # Query blocks of 1024 so exp/mask ops amortize per-op overheads
        # (ACT costs (N+352)/1.2 ns; DVE pays a drain per op).
        NQ2 = 2 * NQ
        mask_pool = ctx.enter_context(tc.tile_pool(name="mask", bufs=1))
        exp_pool = ctx.enter_context(tc.tile_pool(name="exp", bufs=4))
        small = ctx.enter_context(tc.tile_pool(name="small", bufs=2))
        ost_pool = ctx.enter_context(tc.tile_pool(name="ost", bufs=4))
        att_psum = ctx.enter_context(
            tc.tile_pool(name="attps", bufs=1, space="PSUM"))
        scale = 1.0 / np.sqrt(HD)

        for qbp in range(S // NQ2):
            q0 = NQ2 * qbp
            q_sl = slice(q0, q0 + NQ2)
            mt = [mask_pool.tile([128, NQ2], MASK_DT, tag=f"mk{kc}",
                                 name=f"mk{kc}") for kc in range(KC)]
            for kc in range(KC):
                nc.sync.dma_start(
                    mt[kc][:], maskT[128 * kc:128 * (kc + 1), q_sl])
            for hp in range(HPC // 2):
                heads = (2 * hp, 2 * hp + 1)
                pvs = {h: att_psum.tile([HD + 1, NQ2], F32, tag=f"pv{j}",
                                        bufs=1, name=f"pv{j}")
                       for j, h in enumerate(heads)}
                for kc in range(KC):
                    k_sl = slice(128 * kc, 128 * (kc + 1))
                    exs = {}
                    for h in heads:
                        sc = att_psum.tile([128, NQ2], F32, tag="sc",
                                           bufs=2, name="sc")
                        for j in range(2):
                            nc.tensor.matmul(
                                sc[:, NQ * j:NQ * (j + 1)],
                                KT[h][:, k_sl],
                                QT[h][:, q0 + NQ * j:q0 + NQ * (j + 1)],
                                start=True, stop=True)
                        ex = exp_pool.tile([128, NQ2], PV_DT, tag="ex",
                                           name="ex")
                        nc.scalar.activation(ex[:], sc[:], AF.Exp,
                                             scale=scale)
                        nc.vector.tensor_mul(ex[:], ex[:], mt[kc][:])
                        exs[h] = ex
                    for h in heads:
                        v_sl = slice((HD + 1) * h, (HD + 1) * (h + 1))
                        for j in range(2):
                            nc.tensor.matmul(
                                pvs[h][:, NQ * j:NQ * (j + 1)],
                                V[kc][:, v_sl],
                                exs[h][:, NQ * j:NQ * (j + 1)],
                                start=(kc == 0), stop=(kc == KC - 1))
                for h in heads:
                    # epilogue: ho = pv / sums (row 64 of pv). Copy the
                    # sums row to SBUF, broadcast it across 64 partitions
                    # on GpSimd (keeps the PE queue free of blocking tiny
                    # matmuls), then reciprocal at base partition 0 (the
                    # custom-DVE reciprocal mis-executes at base 64).
                    pv = pvs[h]
                    m_i, r0 = (HD * h) // 128, (HD * h) % 128
                    sums_sb = small.tile([65, NQ2], F32, tag="sums",
                                         name="sums_sb")
                    nc.vector.tensor_copy(sums_sb[HD:HD + 1, :],
                                          pv[HD:HD + 1, :])
                    # partition-shifting DMA: gpsimd broadcast reads
                    # physical partition 0
                    sums0 = small.tile([1, NQ2], F32, tag="sums0",
                                       name="sums0")
                    nc.sync.dma_start(sums0[:], sums_sb[HD:HD + 1, :])
                    bc = small.tile([HD, NQ2], F32, tag="bc", name="bc")
                    nc.gpsimd.partition_broadcast(bc[:], sums0[:])
                    nc.vector.reciprocal_approx_fast(bc[:], bc[:])
                    ho_t = small.tile([HD, NQ2], WO_DT, tag="hot",
                                      name="hot")
                    nc.vector.tensor_mul(ho_t[:], pv[0:HD, :], bc[:])
                    nc.sync.dma_start(HO[m_i][r0:r0 + HD, q_sl], ho_t[:])

            # Wo partial projection for this query-block pair
            for dt_ in range(DC):
                ps = att_psum.tile([128, NQ2], F32, tag="sc", bufs=2,
                                   name="wops")
                for j in range(2):
                    for m in range(2):
                        nc.tensor.matmul(
                            ps[:, NQ * j:NQ * (j + 1)],
                            WO[m][:, 128 * dt_:128 * (dt_ + 1)],
                            HO[m][:, q0 + NQ * j:q0 + NQ * (j + 1)],
                            start=(m == 0), stop=(m == 1))
                ost = ost_pool.tile([128, NQ2], F32, tag="ost", name="ost")
                nc.vector.tensor_copy(ost[:], ps[:])
                nc.sync.dma_start(out[128 * dt_:128 * (dt_ + 1), q_sl],
                                  ost[:])
    nc.finalize()
    return nc


def shard_inputs(x, mask, Wq, bq, Wk, bk, Wv, bv, Wo, bo):
    x = np.asarray(x, dtype=np.float32)
    mask = np.asarray(mask)
    xT = [np.ascontiguousarray(x[n].T).astype(IO_NP) for n in range(N)]
    maskT = [np.ascontiguousarray(mask[n, 0].T).astype(MASK_NP)
             for n in range(N)]
    in_maps = []
    for c in range(NCORES):
        n = c // CORES_PER_BATCH
        lo = (c % CORES_PER_BATCH) * DPC
        hi = lo + DPC
        in_maps.append({
            "xT": xT[n],
            "maskT": maskT[n],
            "wq": np.ascontiguousarray(np.asarray(Wq)[:, lo:hi]).astype(IO_NP),
            "wk": np.ascontiguousarray(np.asarray(Wk)[:, lo:hi]).astype(IO_NP),
            "wv": np.ascontiguousarray(np.asarray(Wv)[:, lo:hi]).astype(IO_NP),
            "wo": np.ascontiguousarray(np.asarray(Wo)[lo:hi, :]).astype(WO_NP),
            "bq": np.asarray(bq, dtype=np.float32)[lo:hi].reshape(1, DPC).astype(IO_NP),
            "bk": np.asarray(bk, dtype=np.float32)[lo:hi].reshape(1, DPC).astype(IO_NP),
            "bv": np.asarray(bv, dtype=np.float32)[lo:hi].reshape(1, DPC).astype(IO_NP),
        })
    return in_maps


LAST_RESULTS = None


def kernel(x, mask, Wq, bq, Wk, bk, Wv, bv, Wo, bo):
    global LAST_RESULTS
    with_bias = any(np.any(np.asarray(b)) for b in (bq, bk, bv))
    nc = build_nc(with_bias=with_bias)
    in_maps = shard_inputs(x, mask, Wq, bq, Wk, bk, Wv, bv, Wo, bo)
    if not with_bias:
        for im in in_maps:
            im.pop("bq"), im.pop("bk"), im.pop("bv")
    trace = bool(os.environ.get("ATT_TRACE"))
    res = run_bass_kernel_spmd(nc, in_maps, list(range(NCORES)), trace=trace)
    LAST_RESULTS = res
    outs = [np.asarray(r["out"], dtype=np.float32) for r in res.results]
    y = np.empty((N, S, D), dtype=np.float32)
    bo_f = np.asarray(bo, dtype=np.float32)
    for n in range(N):
        acc = outs[n * CORES_PER_BATCH]
        for c in range(1, CORES_PER_BATCH):
            acc = acc + outs[n * CORES_PER_BATCH + c]
        y[n] = acc.T + bo_f
    return y


# revision 27
# speedup vs baseline: 1.0191x; 1.0191x over previous
"""Multi-head self-attention Trainium2 kernel (8 NeuronCores, SPMD).

Problem: x[2,2048,1024] f32, 16 heads x 64 dim, full QKV+attention+output
projection. Sharding: core = (batch n, head-group of 4 heads). Each core
computes partial^T = Wo_rows^T @ head_out^T for its 4 heads; host sums the
4 partials per batch and transposes back.

Device-side layout is fully "transposed" (feature dim on partitions):
  xT [1024, 2048]  ->  QT/KT [256, 2048] (d' on partitions)
                       V  [2048, 4*65]   (token on partitions, per-head
                                          [V_h | ones] for fused softmax sums)
  scoresT [keys, q] = KT_h^T-slices @ QT_h  (contraction over d=64)
  exp (no max subtraction: scores ~ N(0,1), |s| < ~12 is safe in f32)
  mask applied multiplicatively after exp (masked entries exp*0)
  PV: psum[65, Nq] = V'_h^T @ expT, row 64 = softmax denominators
  out^T/sums -> HO [256, 2048] -> partial^T [1024, 2048] = Wo^T @ HO

Heads are processed in interleaved pairs so the PE always has an
independent matmul chain while the other head's softmax runs (keeps the
HAM clock gate warm). Matmul dtype is float32r (TF32-like, full PE rate).
"""

import os
import sys
import numpy as np

if "/opt/trn_rl_repo" not in sys.path:
    sys.path.insert(0, "/opt/trn_rl_repo")

import ml_dtypes  # noqa: E402
import concourse.bass as bass  # noqa: E402
import concourse.mybir as mybir  # noqa: E402
from concourse import tile  # noqa: E402
from concourse import bacc  # noqa: E402
from concourse.bass_utils import run_bass_kernel_spmd  # noqa: E402
from contextlib import ExitStack  # noqa: E402

F32 = mybir.dt.float32
F32R = mybir.dt.float32r
BF16 = mybir.dt.bfloat16
AF = mybir.ActivationFunctionType

N, S, D = 2, 2048, 1024
H, HD = 16, 64
NCORES = 8
CORES_PER_BATCH = 4
HPC = H // CORES_PER_BATCH      # 4 heads per core
DPC = HPC * HD                  # 256 proj cols per core
NQ = 512                        # query block
NQB = S // NQ                   # 4 query blocks
KC = S // 128                   # 16 key chunks
DC = D // 128                   # 8 contraction chunks of embed dim

# modes: f32r (all f32r), mixed (f32r + bf16 attention weights), bf16, f32
MODE = os.environ.get("ATT_MODE", "mixed")
_d = {
    "f32r":  (F32R, F32R, F32R),
    "mixed": (F32R, BF16, F32R),
    "bf16":  (BF16, BF16, BF16),
    "f32":   (F32, F32, F32),
}
IO_DT, PV_DT, WO_DT = _d[MODE]
IO_NP = ml_dtypes.bfloat16 if IO_DT == BF16 else np.float32
WO_NP = ml_dtypes.bfloat16 if WO_DT == BF16 else np.float32
MASK_DT = BF16
MASK_NP = ml_dtypes.bfloat16
# recip/broadcast stays plain f32: the custom-DVE reciprocal can't tag
# its output as f32r for the BIR verifier, so the rank-1 broadcast matmul
# runs as a plain-f32 matmul (tiny; 4 cyc/row is irrelevant here)
RECIP_DT = F32


def _memset1(nc, ap):
    # DVE memset can't encode f32r; write 1.0 through an f32 view
    # (1.0 is exact in any truncated-mantissa f32 variant)
    if ap.dtype == F32R:
        ap = ap.bitcast(F32)
    nc.vector.memset(ap, 1.0)


def build_nc(with_bias: bool = True) -> bass.Bass:
    nc = bacc.Bacc()
    xT = nc.dram_tensor("xT", [D, S], IO_DT, kind="ExternalInput")
    maskT = nc.dram_tensor("maskT", [S, S], MASK_DT, kind="ExternalInput")
    wq = nc.dram_tensor("wq", [D, DPC], IO_DT, kind="ExternalInput")
    wk = nc.dram_tensor("wk", [D, DPC], IO_DT, kind="ExternalInput")
    wv = nc.dram_tensor("wv", [D, DPC], IO_DT, kind="ExternalInput")
    wo = nc.dram_tensor("wo", [DPC, D], WO_DT, kind="ExternalInput")
    if with_bias:
        bq = nc.dram_tensor("bq", [1, DPC], IO_DT, kind="ExternalInput")
        bk = nc.dram_tensor("bk", [1, DPC], IO_DT, kind="ExternalInput")
        bv = nc.dram_tensor("bv", [1, DPC], IO_DT, kind="ExternalInput")
    out = nc.dram_tensor("out", [D, S], F32, kind="ExternalOutput")

    with tile.TileContext(nc) as tc, ExitStack() as ctx:
        consts = ctx.enter_context(tc.tile_pool(name="consts", bufs=1))
        qkv_pool = ctx.enter_context(tc.tile_pool(name="qkv", bufs=1))

        if with_bias:
            ones_sb = consts.tile([1, S], IO_DT, tag="ones")
            _memset1(nc, ones_sb[:])


        # persistent activations. QT/KT are per-head [128, S] tiles: head
        # h's 64 d'-rows live at their natural partition offset, the other
        # 64 rows are zeroed so scores matmuls contract over K=128 (half-
        # array K=64 matmuls leave the HAM activity monitor cold -> 1.2GHz)
        QT = [qkv_pool.tile([128, S], IO_DT, tag=f"qt{h}", name=f"qt{h}")
              for h in range(HPC)]
        KT = [qkv_pool.tile([128, S], IO_DT, tag=f"kt{h}", name=f"kt{h}")
              for h in range(HPC)]
        for h in range(HPC):
            r0 = (HD * h) % 128
            rz = 64 - r0  # start of the unused half
            zq = QT[h][rz:rz + HD, :]
            zk = KT[h][rz:rz + HD, :]
            if IO_DT == F32R:
                zq, zk = zq.bitcast(F32), zk.bitcast(F32)
            nc.vector.memset(zq, 0.0)
            nc.vector.memset(zk, 0.0)
        V = [qkv_pool.tile([128, HPC * (HD + 1)], PV_DT, tag=f"v{t}",
                           name=f"v{t}") for t in range(KC)]
        HO = [qkv_pool.tile([128, S], WO_DT, tag=f"ho{m}", name=f"ho{m}")
              for m in range(2)]
        WO = [qkv_pool.tile([128, D], WO_DT, tag=f"wo{m}", name=f"wo{m}")
              for m in range(2)]
        for m in range(2):
            nc.sync.dma_start(WO[m][:], wo[128 * m:128 * (m + 1), :])

        # ---- phase 1: projections (scoped pools so SBUF frees after) ----
        with tc.tile_pool(name="ph1", bufs=1) as ph1, \
             tc.tile_pool(name="ph1ps", bufs=2, space="PSUM") as ph1ps:
            xt = [ph1.tile([128, S], IO_DT, tag=f"xt{i}", name=f"xt{i}")
                  for i in range(DC)]
            w_sb = {}
            for wname in ("wq", "wk", "wv"):
                w_sb[wname] = [
                    ph1.tile([128, DPC], IO_DT, tag=f"{wname}{i}",
                             name=f"{wname}{i}") for i in range(DC)]
            for i in range(DC):
                nc.sync.dma_start(xt[i][:], xT[128 * i:128 * (i + 1), :])
                for wname, wdram in (("wq", wq), ("wk", wk), ("wv", wv)):
                    nc.sync.dma_start(w_sb[wname][i][:],
                                      wdram[128 * i:128 * (i + 1), :])
            b_sb = {}
            if with_bias:
                for bname, bdram in (("bq", bq), ("bk", bk), ("bv", bv)):
                    b_sb[bname] = consts.tile([1, DPC], IO_DT, tag=bname,
                                              name=f"{bname}_sb")
                    nc.sync.dma_start(b_sb[bname][:], bdram[:])

            # QT / KT: out[d' tile, tok] = W-chunk^T @ xT-chunk.
            # t-blocks inner with fixed weights so LDWEIGHTS amortizes.
            for dst, wname, bname in ((QT, "wq", "bq"), (KT, "wk", "bk")):
                for m in range(2):
                    pss = [ph1ps.tile([128, NQ], F32, tag=f"projps{t}",
                                      bufs=1, name=f"projps{t}")
                           for t in range(NQB)]
                    for i in range(DC):
                        for t in range(NQB):
                            nc.tensor.matmul(
                                pss[t][:],
                                w_sb[wname][i][:, 128 * m:128 * (m + 1)],
                                xt[i][:, NQ * t:NQ * (t + 1)],
                                start=(i == 0),
                                stop=(not with_bias and i == DC - 1))
                    for t in range(NQB):
                        if with_bias:
                            nc.tensor.matmul(
                                pss[t][:],
                                b_sb[bname][:, 128 * m:128 * (m + 1)],
                                ones_sb[:, NQ * t:NQ * (t + 1)],
                                start=False, stop=True)
                        t_sl = slice(NQ * t, NQ * (t + 1))
                        nc.scalar.copy(dst[2 * m][0:HD, t_sl],
                                       pss[t][0:HD, :])
                        nc.scalar.copy(dst[2 * m + 1][HD:128, t_sl],
                                       pss[t][HD:128, :])

            # V natural: out[tok, d'] = xT-chunk^T(as lhsT) @ Wv-chunk
            for t in range(KC):
                ps = ph1ps.tile([128, DPC], F32, tag="vps", bufs=2,
                                name="vps")
                for i in range(DC):
                    nc.tensor.matmul(
                        ps[:],
                        xt[i][:, 128 * t:128 * (t + 1)],
                        w_sb["wv"][i][:],
                        start=(i == 0),
                        stop=(not with_bias and i == DC - 1))
                if with_bias:
                    nc.tensor.matmul(
                        ps[:], ones_sb[:, 128 * t:128 * (t + 1)],
                        b_sb["bv"][:], start=False, stop=True)
                v3 = V[t].rearrange("p (h d) -> p h d", d=HD + 1)
                nc.scalar.copy(v3[:, :, 0:HD],
                               ps.rearrange("p (h d) -> p h d", d=HD))
                _memset1(nc, v3[:, :, HD:HD + 1])

        # ---- phase 2+3: attention + output projection, per query block ----
        # (pools opened only after phase-1 pools release their SBUF/PSUM)
        # Query blocks of 1024 so exp/mask ops amortize per-op overheads
        # (ACT costs (N+352)/1.2 ns; DVE pays a drain per op).
        NQ2 = 2 * NQ
        mask_pool = ctx.enter_context(tc.tile_pool(name="mask", bufs=1))
        exp_pool = ctx.enter_context(tc.tile_pool(name="exp", bufs=4))
        small = ctx.enter_context(tc.tile_pool(name="small", bufs=2))
        ost_pool = ctx.enter_context(tc.tile_pool(name="ost", bufs=4))
        att_psum = ctx.enter_context(
            tc.tile_pool(name="attps", bufs=1, space="PSUM"))
        scale = 1.0 / np.sqrt(HD)

        # rank-1 broadcast lhsT: ones row at base partition 64 (must
        # match the sums row's base partition)
        ones32 = consts.tile([65, HD], F32, tag="ones32")
        _memset1(nc, ones32[:])

        for qbp in range(S // NQ2):
            q0 = NQ2 * qbp
            q_sl = slice(q0, q0 + NQ2)
            mt = [mask_pool.tile([128, NQ2], MASK_DT, tag=f"mk{kc}",
                                 name=f"mk{kc}") for kc in range(KC)]
            for kc in range(KC):
                nc.sync.dma_start(
                    mt[kc][:], maskT[128 * kc:128 * (kc + 1), q_sl])
            for hp in range(HPC // 2):
                heads = (2 * hp, 2 * hp + 1)
                pvs = {h: att_psum.tile([HD + 1, NQ2], F32, tag=f"pv{j}",
                                        bufs=1, name=f"pv{j}")
                       for j, h in enumerate(heads)}
                for kc in range(KC):
                    k_sl = slice(128 * kc, 128 * (kc + 1))
                    exs = {}
                    for h in heads:
                        sc = att_psum.tile([128, NQ2], F32, tag="sc",
                                           bufs=2, name="sc")
                        for j in range(2):
                            nc.tensor.matmul(
                                sc[:, NQ * j:NQ * (j + 1)],
                                KT[h][:, k_sl],
                                QT[h][:, q0 + NQ * j:q0 + NQ * (j + 1)],
                                start=True, stop=True)
                        ex = exp_pool.tile([128, NQ2], PV_DT, tag="ex",
                                           name="ex")
                        nc.scalar.activation(ex[:], sc[:], AF.Exp,
                                             scale=scale)
                        nc.vector.tensor_mul(ex[:], ex[:], mt[kc][:])
                        exs[h] = ex
                    for h in heads:
                        v_sl = slice((HD + 1) * h, (HD + 1) * (h + 1))
                        for j in range(2):
                            nc.tensor.matmul(
                                pvs[h][:, NQ * j:NQ * (j + 1)],
                                V[kc][:, v_sl],
                                exs[h][:, NQ * j:NQ * (j + 1)],
                                start=(kc == 0), stop=(kc == KC - 1))
                for h in heads:
                    # epilogue: ho = pv / sums (row 64 of pv). Copy the
                    # sums row to SBUF, broadcast it across 64 partitions
                    # with a rank-1 matmul, reciprocal at base partition 0
                    # (the custom-DVE reciprocal mis-executes at base 64),
                    # multiply, store.
                    pv = pvs[h]
                    m_i, r0 = (HD * h) // 128, (HD * h) % 128
                    sums_sb = small.tile([65, NQ2], F32, tag="sums",
                                         name="sums_sb")
                    nc.vector.tensor_copy(sums_sb[HD:HD + 1, :],
                                          pv[HD:HD + 1, :])
                    bc = small.tile([HD, NQ2], F32, tag="bc", name="bc")
                    for j in range(2):
                        big = att_psum.tile([128, NQ2], F32, tag="sc",
                                            bufs=2, name="bcps")
                        bcps = big[0:HD, 0:NQ]
                        nc.tensor.matmul(bcps, ones32[HD:HD + 1, :],
                                         sums_sb[HD:HD + 1,
                                                 NQ * j:NQ * (j + 1)],
                                         start=True, stop=True)
                        nc.scalar.copy(bc[:, NQ * j:NQ * (j + 1)], bcps)
                    nc.vector.reciprocal_approx_fast(bc[:], bc[:])
                    ho_t = small.tile([HD, NQ2], WO_DT, tag="hot",
                                      name="hot")
                    nc.vector.tensor_mul(ho_t[:], pv[0:HD, :], bc[:])
                    nc.sync.dma_start(HO[m_i][r0:r0 + HD, q_sl], ho_t[:])

            # Wo partial projection for this query-block pair
            for dt_ in range(DC):
                ps = att_psum.tile([128, NQ2], F32, tag="sc", bufs=2,
                                   name="wops")
                for j in range(2):
                    for m in range(2):
                        nc.tensor.matmul(
                            ps[:, NQ * j:NQ * (j + 1)],
                            WO[m][:, 128 * dt_:128 * (dt_ + 1)],
                            HO[m][:, q0 + NQ * j:q0 + NQ * (j + 1)],
                            start=(m == 0), stop=(m == 1))
                ost = ost_pool.tile([128, NQ2], F32, tag="ost", name="ost")
                nc.vector.tensor_copy(ost[:], ps[:])
                nc.sync.dma_start(out[128 * dt_:128 * (dt_ + 1), q_sl],
                                  ost[:])
    nc.finalize()
    return nc


def shard_inputs(x, mask, Wq, bq, Wk, bk, Wv, bv, Wo, bo):
    x = np.asarray(x, dtype=np.float32)
    mask = np.asarray(mask)
    xT = [np.ascontiguousarray(x[n].T).astype(IO_NP) for n in range(N)]
    maskT = [np.ascontiguousarray(mask[n, 0].T).astype(MASK_NP)
             for n in range(N)]
    in_maps = []
    for c in range(NCORES):
        n = c // CORES_PER_BATCH
        lo = (c % CORES_PER_BATCH) * DPC
        hi = lo + DPC
        in_maps.append({
            "xT": xT[n],
            "maskT": maskT[n],
            "wq": np.ascontiguousarray(np.asarray(Wq)[:, lo:hi]).astype(IO_NP),
            "wk": np.ascontiguousarray(np.asarray(Wk)[:, lo:hi]).astype(IO_NP),
            "wv": np.ascontiguousarray(np.asarray(Wv)[:, lo:hi]).astype(IO_NP),
            "wo": np.ascontiguousarray(np.asarray(Wo)[lo:hi, :]).astype(WO_NP),
            "bq": np.asarray(bq, dtype=np.float32)[lo:hi].reshape(1, DPC).astype(IO_NP),
            "bk": np.asarray(bk, dtype=np.float32)[lo:hi].reshape(1, DPC).astype(IO_NP),
            "bv": np.asarray(bv, dtype=np.float32)[lo:hi].reshape(1, DPC).astype(IO_NP),
        })
    return in_maps


LAST_RESULTS = None


def kernel(x, mask, Wq, bq, Wk, bk, Wv, bv, Wo, bo):
    global LAST_RESULTS
    with_bias = any(np.any(np.asarray(b)) for b in (bq, bk, bv))
    nc = build_nc(with_bias=with_bias)
    in_maps = shard_inputs(x, mask, Wq, bq, Wk, bk, Wv, bv, Wo, bo)
    if not with_bias:
        for im in in_maps:
            im.pop("bq"), im.pop("bk"), im.pop("bv")
    trace = bool(os.environ.get("ATT_TRACE"))
    res = run_bass_kernel_spmd(nc, in_maps, list(range(NCORES)), trace=trace)
    LAST_RESULTS = res
    outs = [np.asarray(r["out"], dtype=np.float32) for r in res.results]
    y = np.empty((N, S, D), dtype=np.float32)
    bo_f = np.asarray(bo, dtype=np.float32)
    for n in range(N):
        acc = outs[n * CORES_PER_BATCH]
        for c in range(1, CORES_PER_BATCH):
            acc = acc + outs[n * CORES_PER_BATCH + c]
        y[n] = acc.T + bo_f
    return y
